# revision 21
# baseline (speedup 1.0000x reference)
"""AdaAug augmentation pipeline on 8 TRN2 NeuronCores (pure data parallel).

Pipeline per sample: color transform (3x3 + bias) -> 43-tap separable wavelet
filter with reflect padding -> additive RGB noise -> cutout mask.

Strategy (v3):
  - Host derives per-sample small parameters exactly as the reference does:
    color matrix M3/b, filter taps hz', cutout mask field, sigma.  The color
    mix (3x3 + bias, commutes with the linear filter) and the sigma scaling
    are folded into the packing/dtype-conversion pass of the inputs.
  - The separable conv with reflect padding is two chained PE matmul stages
    against a per-sample 256x256 reflect-Toeplitz matrix W.  Each stage uses
    a minimal "banded-2" split per (channel, half): row-block k0 covers
    output cols [0,149), k1 covers [107,256); only the first matmul into a
    PSUM bank uses start=True, later ones ride the per-element has_written
    bits (accumulate where written, overwrite where not).
  - sigma*noise (shipped as fp8e4) is accumulated into the stage-2 PSUM via
    an fp8 identity matmul.
  - Stage-1 eviction: ACT copies.  Final op per channel: out = mask * psum,
    one DVE tensor_tensor, PSUM -> SBUF bf16, then DMA out.
  - All bulk tensors are packed partition-major on the host so every DMA is
    128 contiguous per-partition chunks.
"""

import os
import sys

import numpy as np

if "/opt/trn_rl_repo" not in sys.path:
    sys.path.insert(0, "/opt/trn_rl_repo")

import ml_dtypes

N, C, H, W = 64, 3, 256, 256
NCORES = 8
NLOC = N // NCORES
TAP, PAD = 43, 21
PI = float(np.pi)
BRIGHT_STD, CONTRAST_STD, HUE_MAX, SAT_STD = 0.2, 0.5, 1.0, 1.0
IMGFILTER_STD, NOISE_STD, CUTOUT_SIZE = 1.0, 0.1, 0.5
P_GATE = 1.0
SM = 16  # per-sample slot count in the smalls tensor

BF16 = ml_dtypes.bfloat16

# banded-2 split: W[k0-rows, j] == 0 for j >= 149; W[k1-rows, j] == 0 for
# j < 107 (43-tap band, half-width 21, rows split at 128).
NB0 = 149
NB1 = 107


# --------------------------------------------------------------------------
# Host-side per-sample parameter derivation (mirrors the reference math)
# --------------------------------------------------------------------------

def color_matrices(gates, gauss, unif):
    """Returns M3 [n,3,3] and bvec [n,3] (float64)."""
    g = gates.astype(np.float64)
    ga = gauss.astype(np.float64)
    u = unif.astype(np.float64)
    n = g.shape[0]
    I4 = np.eye(4)
    inv_sqrt3 = 1.0 / np.sqrt(3.0)
    v3 = np.full(3, inv_sqrt3)
    v4 = np.array([inv_sqrt3, inv_sqrt3, inv_sqrt3, 0.0])
    vv = np.outer(v4, v4)

    b = np.where(g[:, 0] < P_GATE, ga[:, 0] * BRIGHT_STD, 0.0)
    T = np.broadcast_to(I4, (n, 4, 4)).copy()
    T[:, :3, 3] = b[:, None]

    c = np.where(g[:, 1] < P_GATE, 2.0 ** (ga[:, 1] * CONTRAST_STD), 1.0)
    S = I4[None] * np.stack([c, c, c, np.ones_like(c)], axis=1)[:, :, None]
    Cm = S @ T

    i_lf = np.floor(u[:, 0] * 2.0)
    i_lf = np.where(g[:, 2] < P_GATE, i_lf, 0.0)
    Cm = (I4[None] - 2.0 * vv[None] * i_lf[:, None, None]) @ Cm

    theta = (u[:, 1] * 2.0 - 1.0) * PI * HUE_MAX
    theta = np.where(g[:, 3] < P_GATE, theta, 0.0)
    I3 = np.eye(3)
    K = np.array([[0.0, -inv_sqrt3, inv_sqrt3],
                  [inv_sqrt3, 0.0, -inv_sqrt3],
                  [-inv_sqrt3, inv_sqrt3, 0.0]])
    co, si = np.cos(theta), np.sin(theta)
    R3 = ((1.0 - co)[:, None, None] * np.outer(v3, v3)[None]
          + co[:, None, None] * I3[None] + si[:, None, None] * K[None])
    R4 = np.broadcast_to(I4, (n, 4, 4)).copy()
    R4[:, :3, :3] = R3
    Cm = R4 @ Cm

    s = np.where(g[:, 4] < P_GATE, 2.0 ** (ga[:, 2] * SAT_STD), 1.0)
    Cm = (vv[None] + (I4 - vv)[None] * s[:, None, None]) @ Cm
    return Cm[:, :3, :3], Cm[:, :3, 3]


def band_taps(gates, gauss, hz_fbank):
    """Returns hz' [n, 43] (float64)."""
    g = gates.astype(np.float64)
    ga = gauss.astype(np.float64)
    fb = hz_fbank.astype(np.float64)
    n = g.shape[0]
    num_bands = fb.shape[0]
    ep = np.array([10.0, 1.0, 1.0, 1.0]) / 13.0
    gg = np.ones((n, num_bands))
    for i in range(num_bands):
        t_i = 2.0 ** (ga[:, 3 + i] * IMGFILTER_STD)
        t_i = np.where(g[:, 5 + i] < P_GATE, t_i, 1.0)
        t = np.ones((n, num_bands))
        t[:, i] = t_i
        t = t / np.sqrt(np.sum(ep * t * t, axis=-1, keepdims=True))
        gg = gg * t
    return gg @ fb


def toeplitz_reflect(k):
    """W [256,256] such that (reflect-pad-21 conv k) == W.T @ x.  k: [43]."""
    Wm = np.zeros((H, H))
    j = np.arange(H)
    for t in range(TAP):
        m = j + t - PAD
        m = np.abs(m)
        m = np.where(m > H - 1, 2 * (H - 1) - m, m)
        np.add.at(Wm, (m, j), k[t])
    return Wm


def mask_field(gates, unif):
    """Exact f32 cutout mask field [n, h, w] (1.0 outside the cutout)."""
    g32 = gates.astype(np.float32)
    u32 = unif.astype(np.float32)
    size = np.where(g32[:, 10] < np.float32(P_GATE),
                    np.float32(CUTOUT_SIZE), np.float32(0.0)).astype(np.float32)
    half = (size * np.float32(0.5)).astype(np.float32)
    coord = ((np.arange(W, dtype=np.float32) + np.float32(0.5))
             / np.float32(W)).astype(np.float32)
    cx, cy = u32[:, 2], u32[:, 3]
    mx = (np.abs(coord[None, :] - cx[:, None]) >= half[:, None]).astype(np.float32)
    my = (np.abs(coord[None, :] - cy[:, None]) >= half[:, None]).astype(np.float32)
    return np.maximum(my[:, :, None], mx[:, None, :])  # [n, h, w]


def derive_params(gates, gauss, unif, hz_fbank):
    """All per-sample derived parameters for the device kernel."""
    n = gates.shape[0]
    M3, bvec = color_matrices(gates, gauss, unif)
    hz = band_taps(gates, gauss, hz_fbank)
    g32 = gates.astype(np.float32)
    sigma = np.where(g32[:, 9] < np.float32(P_GATE),
                     np.abs(gauss[:, 7].astype(np.float32)) * np.float32(NOISE_STD),
                     np.float32(0.0))
    mask = mask_field(gates, unif)
    Wmats = np.stack([toeplitz_reflect(hz[s]) for s in range(n)])
    return dict(M3=M3, bvec=bvec, sigma=sigma, mask=mask, Wmats=Wmats)


def pack_smalls(M3, bvec):
    """[128, n*SM] f32, partition-replicated scalars.

    Slots: 0..8 = M3 row-major (c_out*3 + c_in), 9..11 = bvec."""
    n = M3.shape[0]
    sm = np.zeros((128, n * SM), dtype=np.float32)
    for s in range(n):
        base = s * SM
        sm[:, base:base + 9] = M3[s].reshape(-1).astype(np.float32)[None, :]
        sm[:, base + 9:base + 12] = bvec[s].astype(np.float32)[None, :]
    return sm


def pack_images(x):
    """[n, 3, 256, 256] -> partition-major [n, 128, 1536]:
    buf[s, p, c*512 + t*256 + w] = x[s, c, t*128 + p, w]."""
    n = x.shape[0]
    return np.ascontiguousarray(
        x.reshape(n, C, 2, 128, W).transpose(0, 3, 1, 2, 4).reshape(n, 128, C * 2 * W)
    )


def unpack_images(buf):
    """Inverse of pack_images (for the f32 output)."""
    n = buf.shape[0]
    return np.ascontiguousarray(
        buf.reshape(n, 128, C, 2, W).transpose(0, 2, 3, 1, 4).reshape(n, C, H, W)
    )


def pack_wmats(Wm):
    """[n, 256, 256] -> compact banded [n, 128, 2*NB0]:
    buf[s, p, 0:NB0]       = W[s, p, 0:NB0]          (k0 rows, cols [0,149))
    buf[s, p, NB0:2*NB0]   = W[s, 128+p, NB1:256]    (k1 rows, cols [107,256))
    All other W entries are structurally zero (43-tap band)."""
    n = Wm.shape[0]
    buf = np.empty((n, 128, 2 * NB0), dtype=Wm.dtype)
    buf[:, :, :NB0] = Wm[:, :128, :NB0]
    buf[:, :, NB0:] = Wm[:, 128:, NB1:]
    return np.ascontiguousarray(buf)


def pack_field(f):
    """[n, 256, 256] -> [n, 128, 512]: buf[s, p, t*256 + w] = f[s, t*128+p, w]."""
    n = f.shape[0]
    return np.ascontiguousarray(
        f.reshape(n, 2, 128, W).transpose(0, 2, 1, 3).reshape(n, 128, 2 * W)
    )


# --------------------------------------------------------------------------
# Pure-numpy emulation of the device pipeline (for host-side validation)
# --------------------------------------------------------------------------

def emulate(images, noise_img, params):
    """Emulates the device computation in f64 (no bf16 rounding)."""
    M3, bvec, sigma = params["M3"], params["bvec"], params["sigma"]
    mask, Wmats = params["mask"], params["Wmats"]
    n = images.shape[0]
    out = np.zeros((n, C, H, W), dtype=np.float64)
    for s in range(n):
        Wm = Wmats[s]
        img = images[s].astype(np.float64)
        mixed = np.tensordot(M3[s], img, axes=([1], [0])) + bvec[s][:, None, None]
        st1 = np.tensordot(mixed, Wm, axes=([1], [0]))  # [c, w, h']
        conv = np.tensordot(st1, Wm, axes=([1], [0]))   # [c, h', w']
        conv += sigma[s] * noise_img[s].astype(np.float64)
        out[s] = conv * mask[s][None]
    return out


# --------------------------------------------------------------------------
# Bass kernel builder
# --------------------------------------------------------------------------

def _legalize_waits(nc, max_keep=1):
    """Split multi-semaphore waits into standalone EventSemaphore instructions.

    The deployed walrus accepts at most one sync-wait command per engine
    instruction; Tile emits several. Hoisting extras onto preceding
    EventSemaphore instructions on the same engine queue is semantically
    identical (engines execute their stream in order)."""
    from concourse import mybir
    n_split = 0
    for f in nc.m.functions:
        for blk in f.blocks:
            out = []
            changed = False
            for inst in blk.instructions:
                si = inst.sync_info
                w = list(si.on_wait) if si is not None else []
                if len(w) > max_keep:
                    for extra in w[:-max_keep]:
                        ev = mybir.InstEventSemaphore(
                            name=f"evw_{n_split}", ins=[], outs=[])
                        ev.engine = inst.engine
                        ev.sync_info = mybir.SyncInfo(
                            on_wait=[extra], on_update=[])
                        out.append(ev)
                        n_split += 1
                    inst.sync_info = mybir.SyncInfo(
                        on_wait=w[-max_keep:], on_update=list(si.on_update))
                    changed = True
                out.append(inst)
            if changed:
                blk.instructions = out
    return nc


def _ap_key(arg, extras=()):
    """Identity key for a lowered matmul weights AP."""
    try:
        return (arg.memref, arg.offset, str(arg.ap), str(arg.dtype)) + tuple(
            str(e) for e in extras)
    except AttributeError:
        return None


def _dedupe_ldweights(nc):
    """Drop InstLdweights whose weights AP is identical to the previous weight
    load in the final PE stream (only matmuls/waits in between)."""
    from concourse import mybir
    n_removed = 0
    for f in nc.m.functions:
        for blk in f.blocks:
            out = []
            changed = False
            last_key = None
            for inst in blk.instructions:
                if inst.engine == mybir.EngineType.PE:
                    if isinstance(inst, mybir.InstLdweights):
                        key = _ap_key(
                            inst.ins[0],
                            extras=(inst.perf_mode, inst.is_transpose,
                                    inst.tile_position),
                        )
                        if key is not None and key == last_key:
                            si = inst.sync_info
                            if si is not None and (list(si.on_wait)
                                                   or list(si.on_update)):
                                ev = mybir.InstEventSemaphore(
                                    name=f"ldw_ev_{n_removed}", ins=[], outs=[])
                                ev.engine = inst.engine
                                ev.sync_info = si
                                out.append(ev)
                            n_removed += 1
                            changed = True
                            continue
                        last_key = key
                    elif isinstance(inst, mybir.InstMatmult):
                        if inst.ldweights:
                            last_key = None  # self-loading matmul clobbers
                    elif isinstance(inst, mybir.InstEventSemaphore):
                        pass  # does not touch the PE array
                    else:
                        last_key = None  # unknown PE inst: invalidate
                out.append(inst)
            if changed:
                blk.instructions = out
    return n_removed


def _fuse_ldweights(nc):
    """Fuse each standalone InstLdweights into its immediately-following
    InstMatmult (self-loading matmul).  The matmul already carries the
    stationary AP in ins[1]; the LDW's sync waits/updates are preserved on
    a standalone EventSemaphore.  Self-loading matmuls are what walrus'
    LDW optimization (background weight-buffer double-buffering) accepts."""
    from concourse import mybir
    n_fused = 0
    for f in nc.m.functions:
        for blk in f.blocks:
            out = []
            pending = None  # (ldw_inst)
            for inst in blk.instructions:
                if inst.engine == mybir.EngineType.PE and isinstance(
                        inst, mybir.InstLdweights):
                    if pending is not None:
                        out.append(pending)  # two LDW in a row: keep first
                    pending = inst
                    continue
                if pending is not None and isinstance(inst, mybir.InstMatmult):
                    si = pending.sync_info
                    waits = list(si.on_wait) if si is not None else []
                    upds = list(si.on_update) if si is not None else []
                    if waits:
                        # LDW waits must still gate the fused matmul
                        ev = mybir.InstEventSemaphore(
                            name=f"ldwf_ev_{n_fused}", ins=[], outs=[])
                        ev.engine = pending.engine
                        ev.sync_info = mybir.SyncInfo(
                            on_wait=waits, on_update=[])
                        out.append(ev)
                    if upds:
                        # LDW updates signal the weights-read complete: they
                        # must fire with the matmul, not before it
                        msi = inst.sync_info
                        mw = list(msi.on_wait) if msi is not None else []
                        mu = list(msi.on_update) if msi is not None else []
                        inst.sync_info = mybir.SyncInfo(
                            on_wait=mw, on_update=mu + upds)
                    inst.ldweights = True
                    n_fused += 1
                    pending = None
                elif pending is not None and inst.engine == mybir.EngineType.PE:
                    out.append(pending)  # non-matmul PE inst: keep LDW as-is
                    pending = None
                out.append(inst)
            if pending is not None:
                out.append(pending)
            blk.instructions = out
    return n_fused


def build_bass(legalize=True, dedupe_ldw=True, fuse_ldw=True):
    import concourse.bass as bass
    import concourse.tile as tile
    from concourse import mybir

    f32 = mybir.dt.float32
    bf16 = mybir.dt.bfloat16
    fp8 = mybir.dt.float8e4
    Alu = mybir.AluOpType
    Act = mybir.ActivationFunctionType

    nc = bass.Bass()
    d_img = nc.declare_dram_parameter("imgs", [NLOC, 128, C * 2 * W], bf16,
                                      isOutput=False)
    d_noi = nc.declare_dram_parameter("noise", [NLOC, 128, C * 2 * W], fp8,
                                      isOutput=False)
    d_w = nc.declare_dram_parameter("wmat", [NLOC, 128, 2 * NB0], bf16,
                                    isOutput=False)
    d_msk = nc.declare_dram_parameter("maskf", [NLOC, 128, 2 * W], fp8,
                                      isOutput=False)
    d_id = nc.declare_dram_parameter("consts", [128, 128], fp8, isOutput=False)
    d_out = nc.declare_dram_parameter("out", [NLOC, 128, C * 2 * W], bf16,
                                      isOutput=True)

    with tile.TileContext(nc) as tc:
        with (
            tc.tile_pool(name="singles", bufs=1) as singles,
            tc.tile_pool(name="io", bufs=3) as io,
            tc.tile_pool(name="work", bufs=3) as work,
            tc.tile_pool(name="ps1", bufs=4, space="PSUM") as ps1p,
            tc.tile_pool(name="ps2", bufs=4, space="PSUM") as ps2p,
        ):
            ident = singles.tile([128, 128], fp8)
            nc.sync.dma_start(out=ident, in_=d_id[:])

            def conv_stage(pt, lhs, w_sb, last_stop, first_start=True):
                """pt[:, m*256+j] += sum_h lhs(k,m)[:,h] W[k*128+h, j].

                banded-2: k0 covers j in [0,NB0), k1 covers [NB1,256).
                Only the very first matmul into the bank uses start=True;
                the rest ride per-element has_written bits."""
                for m in range(2):
                    nc.tensor.matmul(
                        pt[:, m * 256: m * 256 + NB0],
                        lhs(0, m), w_sb[:, 0:NB0],
                        start=(first_start and m == 0), stop=False,
                        skip_group_check=True)
                    nc.tensor.matmul(
                        pt[:, m * 256 + NB1: (m + 1) * 256],
                        lhs(1, m), w_sb[:, NB0: 2 * NB0],
                        start=False,
                        stop=(last_stop and m == 1),
                        skip_group_check=True)

            def ch(t, c):
                return t[:, c * 512:(c + 1) * 512]

            # software pipeline: stage A(s) = loads + stage-1 + evict;
            # stage B(s) = noise + stage-2 + finals + store.  B(s-1) is
            # emitted between A(s-1) and A(s) so the PE always has dense
            # work while ACT evicts (keeps the HAM clock-gate warm).
            stash = {}
            for it in range(NLOC + 1):
                if it < NLOC:
                    s = it
                    # ---- loads (all contiguous per partition) ----
                    img_sb = io.tile([128, C * 2 * W], bf16, tag="img")
                    nc.sync.dma_start(out=img_sb, in_=d_img[s])
                    w_sb = io.tile([128, 2 * NB0], bf16, tag="wm")
                    nc.sync.dma_start(out=w_sb, in_=d_w[s])
                    noi_sb = io.tile([128, C * 2 * W], fp8, tag="noi")
                    nc.sync.dma_start(out=noi_sb, in_=d_noi[s])
                    msk_sb = io.tile([128, 2 * W], fp8, tag="msk")
                    nc.sync.dma_start(out=msk_sb, in_=d_msk[s])

                    # ---- stage 1: vertical conv per channel ----
                    # ps1_c[p, m*256+j] = sum_h img[c,h,w=m*128+p] * W[h,j]
                    # (img arrives color-premixed from the host pack pass)
                    out1 = work.tile([128, C * 2 * H], bf16, tag="out1")
                    for c in range(C):
                        pt = ps1p.tile([128, 2 * H], f32, tag="ps1",
                                       name=f"ps1_{s}_{c}")
                        conv_stage(
                            pt,
                            lambda k, m, _c=c, _t=img_sb:
                                img_like_slice(_t, _c, k, m),
                            w_sb, last_stop=True)
                        # evict PSUM -> SBUF bf16 (stage-2 stationary)
                        nc.scalar.copy(ch(out1, c), pt)
                    stash[s] = (w_sb, noi_sb, msk_sb, out1)

                if it >= 1:
                    s = it - 1
                    w_sb, noi_sb, msk_sb, out1 = stash.pop(s)
                    # ---- stage 2: noise first (sets all has_written bits
                    # with start=True), then the conv accumulates on top ----
                    outS = work.tile([128, C * 2 * W], bf16, tag="outS")
                    pt2s = []
                    for c in range(C):
                        pt2 = ps2p.tile([128, 2 * W], f32, tag="ps2",
                                        name=f"ps2_{s}_{c}")
                        # pt2 = I @ (sigma*noise)_c   (fp8 identity; the 3
                        # consecutive ident loads dedupe to one LDWEIGHTS)
                        nc.tensor.matmul(
                            pt2[:], ident[:], ch(noi_sb, c),
                            start=True, stop=False, skip_group_check=True)
                        pt2s.append(pt2)
                    for c in range(C):
                        conv_stage(
                            pt2s[c],
                            lambda k, m, _c=c, _t=out1:
                                img_like_slice(_t, _c, k, m),
                            w_sb, last_stop=True, first_start=False)
                        # final: out = mask * (conv + noise)
                        nc.vector.tensor_tensor(
                            ch(outS, c), msk_sb[:], pt2s[c][:], Alu.mult)
                    nc.sync.dma_start(out=d_out[s], in_=outS)

    if dedupe_ldw:
        n = _dedupe_ldweights(nc)
        if os.environ.get("ADAAUG_DEBUG"):
            print(f"deduped {n} LDWEIGHTS")
    if fuse_ldw:
        n = _fuse_ldweights(nc)
        if os.environ.get("ADAAUG_DEBUG"):
            print(f"fused {n} LDWEIGHTS")
    return _legalize_waits(nc) if legalize else nc


def img_like_slice(t, c, k, m):
    """[128,128] lhsT tile: rows h in k-block, cols w/j in m-block of ch c."""
    base = c * 512 + k * 256 + m * 128
    return t[:, base: base + 128]


# --------------------------------------------------------------------------
# Entry point
# --------------------------------------------------------------------------

def _prep_in_maps(images, gates, gauss, unif, noise_img, hz_fbank):
    from concourse import mybir

    prm = derive_params(gates, gauss, unif, hz_fbank)
    fp8_np = mybir.dt.np(mybir.dt.float8e4)
    # color premix folded into the packing/conversion pass:
    # mixed[s,c] = sum_c' M3[s,c,c'] img[s,c'] + b[s,c]
    mixed = np.einsum(
        "scd,sdhw->schw", prm["M3"].astype(np.float32),
        images.astype(np.float32)) + prm["bvec"].astype(np.float32)[:, :, None, None]
    imgs_bf = pack_images(mixed).astype(BF16)
    noiseS = noise_img * prm["sigma"][:, None, None, None].astype(np.float32)
    noise_f8 = pack_images(noiseS).astype(fp8_np)
    w_bf = pack_wmats(prm["Wmats"].astype(np.float32)).astype(BF16)
    mask_bf = pack_field(prm["mask"]).astype(fp8_np)
    ident_f8 = np.eye(128, dtype=np.float32).astype(fp8_np)
    in_maps = []
    for i in range(NCORES):
        lo, hi = i * NLOC, (i + 1) * NLOC
        in_maps.append({
            "imgs": np.ascontiguousarray(imgs_bf[lo:hi]),
            "noise": np.ascontiguousarray(noise_f8[lo:hi]),
            "wmat": np.ascontiguousarray(w_bf[lo:hi]),
            "maskf": np.ascontiguousarray(mask_bf[lo:hi]),
            "consts": ident_f8,
        })
    return in_maps, prm


_NC_CACHE = {}


def run_on_hw(images, gates, gauss, unif, noise_img, hz_fbank, trace=False):
    from concourse.bass_utils import run_bass_kernel_spmd

    if "nc" not in _NC_CACHE:
        _NC_CACHE["nc"] = build_bass(
            dedupe_ldw=os.environ.get("ADAAUG_DEDUPE_LDW", "0") == "1")
    nc = _NC_CACHE["nc"]
    in_maps, _ = _prep_in_maps(images, gates, gauss, unif, noise_img, hz_fbank)
    res = run_bass_kernel_spmd(
        nc, in_maps, core_ids=list(range(NCORES)), trace=trace
    )
    out = np.concatenate(
        [unpack_images(np.asarray(r["out"]).astype(np.float32))
         for r in res.results], axis=0
    )
    return out.astype(np.float32), res


def kernel(images, gates, gauss, unif, noise_img, hz_fbank):
    images = np.asarray(images, dtype=np.float32)
    gates = np.asarray(gates, dtype=np.float32)
    gauss = np.asarray(gauss, dtype=np.float32)
    unif = np.asarray(unif, dtype=np.float32)
    noise_img = np.asarray(noise_img, dtype=np.float32)
    hz_fbank = np.asarray(hz_fbank, dtype=np.float32)
    out, _ = run_on_hw(images, gates, gauss, unif, noise_img, hz_fbank,
                       trace=os.environ.get("ADAAUG_TRACE", "0") == "1")
    return out


# revision 27
# speedup vs baseline: 1.0966x; 1.0966x over previous
"""AdaAug augmentation pipeline on 8 TRN2 NeuronCores (pure data parallel).

Pipeline per sample: color transform (3x3 + bias) -> 43-tap separable wavelet
filter with reflect padding -> additive RGB noise -> cutout mask.

Strategy (v3):
  - Host derives per-sample small parameters exactly as the reference does:
    color matrix M3/b, filter taps hz', cutout mask field, sigma.  The color
    mix (3x3 + bias, commutes with the linear filter) and the sigma scaling
    are folded into the packing/dtype-conversion pass of the inputs.
  - The separable conv with reflect padding is two chained PE matmul stages
    against a per-sample 256x256 reflect-Toeplitz matrix W.  Each stage uses
    a minimal "banded-2" split per (channel, half): row-block k0 covers
    output cols [0,149), k1 covers [107,256); only the first matmul into a
    PSUM bank uses start=True, later ones ride the per-element has_written
    bits (accumulate where written, overwrite where not).
  - sigma*noise (shipped as fp8e4) is accumulated into the stage-2 PSUM via
    an fp8 identity matmul.
  - Stage-1 eviction: ACT copies.  Final op per channel: out = mask * psum,
    one DVE tensor_tensor, PSUM -> SBUF bf16, then DMA out.
  - All bulk tensors are packed partition-major on the host so every DMA is
    128 contiguous per-partition chunks.
"""

import os
import sys

import numpy as np

if "/opt/trn_rl_repo" not in sys.path:
    sys.path.insert(0, "/opt/trn_rl_repo")

import ml_dtypes

N, C, H, W = 64, 3, 256, 256
NCORES = 8
NLOC = N // NCORES
TAP, PAD = 43, 21
PI = float(np.pi)
BRIGHT_STD, CONTRAST_STD, HUE_MAX, SAT_STD = 0.2, 0.5, 1.0, 1.0
IMGFILTER_STD, NOISE_STD, CUTOUT_SIZE = 1.0, 0.1, 0.5
P_GATE = 1.0
SM = 16  # per-sample slot count in the smalls tensor

BF16 = ml_dtypes.bfloat16

# banded-2 split: W[k0-rows, j] == 0 for j >= 149; W[k1-rows, j] == 0 for
# j < 107 (43-tap band, half-width 21, rows split at 128).
NB0 = 149
NB1 = 107

# packed per-sample input row: img bf16 | noise fp8 | W bf16 | mask fp8
OFF_IMG = 0
OFF_NOI = OFF_IMG + C * 2 * W * 2          # 3072
OFF_W = OFF_NOI + C * 2 * W                # 4608
OFF_MSK = OFF_W + 2 * NB0 * 2              # 5204
INB = OFF_MSK + 2 * W                      # 5716 bytes per partition


# --------------------------------------------------------------------------
# Host-side per-sample parameter derivation (mirrors the reference math)
# --------------------------------------------------------------------------

def color_matrices(gates, gauss, unif):
    """Returns M3 [n,3,3] and bvec [n,3] (float64)."""
    g = gates.astype(np.float64)
    ga = gauss.astype(np.float64)
    u = unif.astype(np.float64)
    n = g.shape[0]
    I4 = np.eye(4)
    inv_sqrt3 = 1.0 / np.sqrt(3.0)
    v3 = np.full(3, inv_sqrt3)
    v4 = np.array([inv_sqrt3, inv_sqrt3, inv_sqrt3, 0.0])
    vv = np.outer(v4, v4)

    b = np.where(g[:, 0] < P_GATE, ga[:, 0] * BRIGHT_STD, 0.0)
    T = np.broadcast_to(I4, (n, 4, 4)).copy()
    T[:, :3, 3] = b[:, None]

    c = np.where(g[:, 1] < P_GATE, 2.0 ** (ga[:, 1] * CONTRAST_STD), 1.0)
    S = I4[None] * np.stack([c, c, c, np.ones_like(c)], axis=1)[:, :, None]
    Cm = S @ T

    i_lf = np.floor(u[:, 0] * 2.0)
    i_lf = np.where(g[:, 2] < P_GATE, i_lf, 0.0)
    Cm = (I4[None] - 2.0 * vv[None] * i_lf[:, None, None]) @ Cm

    theta = (u[:, 1] * 2.0 - 1.0) * PI * HUE_MAX
    theta = np.where(g[:, 3] < P_GATE, theta, 0.0)
    I3 = np.eye(3)
    K = np.array([[0.0, -inv_sqrt3, inv_sqrt3],
                  [inv_sqrt3, 0.0, -inv_sqrt3],
                  [-inv_sqrt3, inv_sqrt3, 0.0]])
    co, si = np.cos(theta), np.sin(theta)
    R3 = ((1.0 - co)[:, None, None] * np.outer(v3, v3)[None]
          + co[:, None, None] * I3[None] + si[:, None, None] * K[None])
    R4 = np.broadcast_to(I4, (n, 4, 4)).copy()
    R4[:, :3, :3] = R3
    Cm = R4 @ Cm

    s = np.where(g[:, 4] < P_GATE, 2.0 ** (ga[:, 2] * SAT_STD), 1.0)
    Cm = (vv[None] + (I4 - vv)[None] * s[:, None, None]) @ Cm
    return Cm[:, :3, :3], Cm[:, :3, 3]


def band_taps(gates, gauss, hz_fbank):
    """Returns hz' [n, 43] (float64)."""
    g = gates.astype(np.float64)
    ga = gauss.astype(np.float64)
    fb = hz_fbank.astype(np.float64)
    n = g.shape[0]
    num_bands = fb.shape[0]
    ep = np.array([10.0, 1.0, 1.0, 1.0]) / 13.0
    gg = np.ones((n, num_bands))
    for i in range(num_bands):
        t_i = 2.0 ** (ga[:, 3 + i] * IMGFILTER_STD)
        t_i = np.where(g[:, 5 + i] < P_GATE, t_i, 1.0)
        t = np.ones((n, num_bands))
        t[:, i] = t_i
        t = t / np.sqrt(np.sum(ep * t * t, axis=-1, keepdims=True))
        gg = gg * t
    return gg @ fb


def toeplitz_reflect(k):
    """W [256,256] such that (reflect-pad-21 conv k) == W.T @ x.  k: [43]."""
    Wm = np.zeros((H, H))
    j = np.arange(H)
    for t in range(TAP):
        m = j + t - PAD
        m = np.abs(m)
        m = np.where(m > H - 1, 2 * (H - 1) - m, m)
        np.add.at(Wm, (m, j), k[t])
    return Wm


def mask_field(gates, unif):
    """Exact f32 cutout mask field [n, h, w] (1.0 outside the cutout)."""
    g32 = gates.astype(np.float32)
    u32 = unif.astype(np.float32)
    size = np.where(g32[:, 10] < np.float32(P_GATE),
                    np.float32(CUTOUT_SIZE), np.float32(0.0)).astype(np.float32)
    half = (size * np.float32(0.5)).astype(np.float32)
    coord = ((np.arange(W, dtype=np.float32) + np.float32(0.5))
             / np.float32(W)).astype(np.float32)
    cx, cy = u32[:, 2], u32[:, 3]
    mx = (np.abs(coord[None, :] - cx[:, None]) >= half[:, None]).astype(np.float32)
    my = (np.abs(coord[None, :] - cy[:, None]) >= half[:, None]).astype(np.float32)
    return np.maximum(my[:, :, None], mx[:, None, :])  # [n, h, w]


def derive_params(gates, gauss, unif, hz_fbank):
    """All per-sample derived parameters for the device kernel."""
    n = gates.shape[0]
    M3, bvec = color_matrices(gates, gauss, unif)
    hz = band_taps(gates, gauss, hz_fbank)
    g32 = gates.astype(np.float32)
    sigma = np.where(g32[:, 9] < np.float32(P_GATE),
                     np.abs(gauss[:, 7].astype(np.float32)) * np.float32(NOISE_STD),
                     np.float32(0.0))
    mask = mask_field(gates, unif)
    Wmats = np.stack([toeplitz_reflect(hz[s]) for s in range(n)])
    return dict(M3=M3, bvec=bvec, sigma=sigma, mask=mask, Wmats=Wmats)


def pack_smalls(M3, bvec):
    """[128, n*SM] f32, partition-replicated scalars.

    Slots: 0..8 = M3 row-major (c_out*3 + c_in), 9..11 = bvec."""
    n = M3.shape[0]
    sm = np.zeros((128, n * SM), dtype=np.float32)
    for s in range(n):
        base = s * SM
        sm[:, base:base + 9] = M3[s].reshape(-1).astype(np.float32)[None, :]
        sm[:, base + 9:base + 12] = bvec[s].astype(np.float32)[None, :]
    return sm


def pack_images(x):
    """[n, 3, 256, 256] -> partition-major [n, 128, 1536]:
    buf[s, p, c*512 + t*256 + w] = x[s, c, t*128 + p, w]."""
    n = x.shape[0]
    return np.ascontiguousarray(
        x.reshape(n, C, 2, 128, W).transpose(0, 3, 1, 2, 4).reshape(n, 128, C * 2 * W)
    )


def unpack_images(buf):
    """Inverse of pack_images (for the f32 output)."""
    n = buf.shape[0]
    return np.ascontiguousarray(
        buf.reshape(n, 128, C, 2, W).transpose(0, 2, 3, 1, 4).reshape(n, C, H, W)
    )


def pack_wmats(Wm):
    """[n, 256, 256] -> compact banded [n, 128, 2*NB0]:
    buf[s, p, 0:NB0]       = W[s, p, 0:NB0]          (k0 rows, cols [0,149))
    buf[s, p, NB0:2*NB0]   = W[s, 128+p, NB1:256]    (k1 rows, cols [107,256))
    All other W entries are structurally zero (43-tap band)."""
    n = Wm.shape[0]
    buf = np.empty((n, 128, 2 * NB0), dtype=Wm.dtype)
    buf[:, :, :NB0] = Wm[:, :128, :NB0]
    buf[:, :, NB0:] = Wm[:, 128:, NB1:]
    return np.ascontiguousarray(buf)


def pack_field(f):
    """[n, 256, 256] -> [n, 128, 512]: buf[s, p, t*256 + w] = f[s, t*128+p, w]."""
    n = f.shape[0]
    return np.ascontiguousarray(
        f.reshape(n, 2, 128, W).transpose(0, 2, 1, 3).reshape(n, 128, 2 * W)
    )


# --------------------------------------------------------------------------
# Pure-numpy emulation of the device pipeline (for host-side validation)
# --------------------------------------------------------------------------

def emulate(images, noise_img, params):
    """Emulates the device computation in f64 (no bf16 rounding)."""
    M3, bvec, sigma = params["M3"], params["bvec"], params["sigma"]
    mask, Wmats = params["mask"], params["Wmats"]
    n = images.shape[0]
    out = np.zeros((n, C, H, W), dtype=np.float64)
    for s in range(n):
        Wm = Wmats[s]
        img = images[s].astype(np.float64)
        mixed = np.tensordot(M3[s], img, axes=([1], [0])) + bvec[s][:, None, None]
        st1 = np.tensordot(mixed, Wm, axes=([1], [0]))  # [c, w, h']
        conv = np.tensordot(st1, Wm, axes=([1], [0]))   # [c, h', w']
        conv += sigma[s] * noise_img[s].astype(np.float64)
        out[s] = conv * mask[s][None]
    return out


# --------------------------------------------------------------------------
# Bass kernel builder
# --------------------------------------------------------------------------

def _legalize_waits(nc, max_keep=1):
    """Split multi-semaphore waits into standalone EventSemaphore instructions.

    The deployed walrus accepts at most one sync-wait command per engine
    instruction; Tile emits several. Hoisting extras onto preceding
    EventSemaphore instructions on the same engine queue is semantically
    identical (engines execute their stream in order)."""
    from concourse import mybir
    n_split = 0
    for f in nc.m.functions:
        for blk in f.blocks:
            out = []
            changed = False
            for inst in blk.instructions:
                si = inst.sync_info
                w = list(si.on_wait) if si is not None else []
                if len(w) > max_keep:
                    for extra in w[:-max_keep]:
                        ev = mybir.InstEventSemaphore(
                            name=f"evw_{n_split}", ins=[], outs=[])
                        ev.engine = inst.engine
                        ev.sync_info = mybir.SyncInfo(
                            on_wait=[extra], on_update=[])
                        out.append(ev)
                        n_split += 1
                    inst.sync_info = mybir.SyncInfo(
                        on_wait=w[-max_keep:], on_update=list(si.on_update))
                    changed = True
                out.append(inst)
            if changed:
                blk.instructions = out
    return nc


def _ap_key(arg, extras=()):
    """Identity key for a lowered matmul weights AP."""
    try:
        return (arg.memref, arg.offset, str(arg.ap), str(arg.dtype)) + tuple(
            str(e) for e in extras)
    except AttributeError:
        return None


def _dedupe_ldweights(nc):
    """Drop InstLdweights whose weights AP is identical to the previous weight
    load in the final PE stream (only matmuls/waits in between)."""
    from concourse import mybir
    n_removed = 0
    for f in nc.m.functions:
        for blk in f.blocks:
            out = []
            changed = False
            last_key = None
            for inst in blk.instructions:
                if inst.engine == mybir.EngineType.PE:
                    if isinstance(inst, mybir.InstLdweights):
                        key = _ap_key(
                            inst.ins[0],
                            extras=(inst.perf_mode, inst.is_transpose,
                                    inst.tile_position),
                        )
                        if key is not None and key == last_key:
                            si = inst.sync_info
                            if si is not None and (list(si.on_wait)
                                                   or list(si.on_update)):
                                ev = mybir.InstEventSemaphore(
                                    name=f"ldw_ev_{n_removed}", ins=[], outs=[])
                                ev.engine = inst.engine
                                ev.sync_info = si
                                out.append(ev)
                            n_removed += 1
                            changed = True
                            continue
                        last_key = key
                    elif isinstance(inst, mybir.InstMatmult):
                        if inst.ldweights:
                            last_key = None  # self-loading matmul clobbers
                    elif isinstance(inst, mybir.InstEventSemaphore):
                        pass  # does not touch the PE array
                    else:
                        last_key = None  # unknown PE inst: invalidate
                out.append(inst)
            if changed:
                blk.instructions = out
    return n_removed


def _fuse_ldweights(nc):
    """Fuse each standalone InstLdweights into its immediately-following
    InstMatmult (self-loading matmul).  The matmul already carries the
    stationary AP in ins[1]; the LDW's sync waits/updates are preserved on
    a standalone EventSemaphore.  Self-loading matmuls are what walrus'
    LDW optimization (background weight-buffer double-buffering) accepts."""
    from concourse import mybir
    n_fused = 0
    for f in nc.m.functions:
        for blk in f.blocks:
            out = []
            pending = None  # (ldw_inst)
            for inst in blk.instructions:
                if inst.engine == mybir.EngineType.PE and isinstance(
                        inst, mybir.InstLdweights):
                    if pending is not None:
                        out.append(pending)  # two LDW in a row: keep first
                    pending = inst
                    continue
                if pending is not None and isinstance(inst, mybir.InstMatmult):
                    si = pending.sync_info
                    waits = list(si.on_wait) if si is not None else []
                    upds = list(si.on_update) if si is not None else []
                    if waits:
                        # LDW waits must still gate the fused matmul
                        ev = mybir.InstEventSemaphore(
                            name=f"ldwf_ev_{n_fused}", ins=[], outs=[])
                        ev.engine = pending.engine
                        ev.sync_info = mybir.SyncInfo(
                            on_wait=waits, on_update=[])
                        out.append(ev)
                    if upds:
                        # LDW updates signal the weights-read complete: they
                        # must fire with the matmul, not before it
                        msi = inst.sync_info
                        mw = list(msi.on_wait) if msi is not None else []
                        mu = list(msi.on_update) if msi is not None else []
                        inst.sync_info = mybir.SyncInfo(
                            on_wait=mw, on_update=mu + upds)
                    inst.ldweights = True
                    n_fused += 1
                    pending = None
                elif pending is not None and inst.engine == mybir.EngineType.PE:
                    out.append(pending)  # non-matmul PE inst: keep LDW as-is
                    pending = None
                out.append(inst)
            if pending is not None:
                out.append(pending)
            blk.instructions = out
    return n_fused


def build_bass(legalize=True, dedupe_ldw=True, fuse_ldw=True):
    import concourse.bass as bass
    import concourse.tile as tile
    from concourse import mybir

    f32 = mybir.dt.float32
    bf16 = mybir.dt.bfloat16
    fp8 = mybir.dt.float8e4
    Alu = mybir.AluOpType
    Act = mybir.ActivationFunctionType

    nc = bass.Bass()
    u8 = mybir.dt.uint8
    d_in = nc.declare_dram_parameter("inpack", [NLOC, 128, INB], u8,
                                     isOutput=False)
    d_id = nc.declare_dram_parameter("consts", [128, 128], fp8, isOutput=False)
    d_out = nc.declare_dram_parameter("out", [NLOC, 128, C * 2 * W], bf16,
                                      isOutput=True)

    with tile.TileContext(nc) as tc:
        with (
            tc.tile_pool(name="singles", bufs=1) as singles,
            tc.tile_pool(name="io", bufs=NLOC) as io,
            tc.tile_pool(name="work", bufs=3) as work,
            tc.tile_pool(name="ps1", bufs=4, space="PSUM") as ps1p,
            tc.tile_pool(name="ps2", bufs=4, space="PSUM") as ps2p,
        ):
            ident = singles.tile([128, 128], fp8)
            nc.sync.dma_start(out=ident, in_=d_id[:])

            def conv_stage(pt, lhs, w_sb, last_stop, first_start=True):
                """pt[:, m*256+j] += sum_h lhs(k,m)[:,h] W[k*128+h, j].

                banded-2: k0 covers j in [0,NB0), k1 covers [NB1,256).
                Only the very first matmul into the bank uses start=True;
                the rest ride per-element has_written bits."""
                for m in range(2):
                    nc.tensor.matmul(
                        pt[:, m * 256: m * 256 + NB0],
                        lhs(0, m), w_sb[:, 0:NB0],
                        start=(first_start and m == 0), stop=False,
                        skip_group_check=True)
                    nc.tensor.matmul(
                        pt[:, m * 256 + NB1: (m + 1) * 256],
                        lhs(1, m), w_sb[:, NB0: 2 * NB0],
                        start=False,
                        stop=(last_stop and m == 1),
                        skip_group_check=True)

            def ch(t, c):
                return t[:, c * 512:(c + 1) * 512]

            # software pipeline: stage A(s) = loads + stage-1 + evict;
            # stage B(s) = noise + stage-2 + finals + store.  B(s-1) is
            # emitted between A(s-1) and A(s) so the PE always has dense
            # work while ACT evicts (keeps the HAM clock-gate warm).
            stash = {}
            for it in range(NLOC + 1):
                if it < NLOC:
                    s = it
                    # ---- one packed load per sample (contiguous rows) ----
                    in_sb = io.tile([128, INB], mybir.dt.uint8, tag="in")
                    nc.sync.dma_start(out=in_sb, in_=d_in[s])
                    img_sb = in_sb[:, OFF_IMG:OFF_NOI].bitcast(bf16)
                    noi_sb = in_sb[:, OFF_NOI:OFF_W].bitcast(fp8)
                    w_sb = in_sb[:, OFF_W:OFF_MSK].bitcast(bf16)
                    msk_sb = in_sb[:, OFF_MSK:INB].bitcast(fp8)

                    # ---- stage 1: vertical conv per channel ----
                    # ps1_c[p, m*256+j] = sum_h img[c,h,w=m*128+p] * W[h,j]
                    # (img arrives color-premixed from the host pack pass)
                    out1 = work.tile([128, C * 2 * H], bf16, tag="out1")
                    for c in range(C):
                        pt = ps1p.tile([128, 2 * H], f32, tag="ps1",
                                       name=f"ps1_{s}_{c}")
                        conv_stage(
                            pt,
                            lambda k, m, _c=c, _t=img_sb:
                                img_like_slice(_t, _c, k, m),
                            w_sb, last_stop=True)
                        # evict PSUM -> SBUF bf16 (stage-2 stationary)
                        nc.scalar.copy(ch(out1, c), pt)
                    stash[s] = (w_sb, noi_sb, msk_sb, out1)

                if it >= 1:
                    s = it - 1
                    w_sb, noi_sb, msk_sb, out1 = stash.pop(s)
                    # ---- stage 2: noise first (sets all has_written bits
                    # with start=True), then the conv accumulates on top ----
                    outS = work.tile([128, C * 2 * W], bf16, tag="outS")
                    pt2s = []
                    for c in range(C):
                        pt2 = ps2p.tile([128, 2 * W], f32, tag="ps2",
                                        name=f"ps2_{s}_{c}")
                        # pt2 = I @ (sigma*noise)_c   (fp8 identity; the 3
                        # consecutive ident loads dedupe to one LDWEIGHTS)
                        nc.tensor.matmul(
                            pt2[:], ident[:], ch(noi_sb, c),
                            start=True, stop=False, skip_group_check=True)
                        pt2s.append(pt2)
                    for c in range(C):
                        conv_stage(
                            pt2s[c],
                            lambda k, m, _c=c, _t=out1:
                                img_like_slice(_t, _c, k, m),
                            w_sb, last_stop=True, first_start=False)
                        # final: out = mask * (conv + noise)
                        nc.vector.tensor_tensor(
                            ch(outS, c), msk_sb, pt2s[c][:], Alu.mult)
                    nc.sync.dma_start(out=d_out[s], in_=outS)

    if dedupe_ldw:
        n = _dedupe_ldweights(nc)
        if os.environ.get("ADAAUG_DEBUG"):
            print(f"deduped {n} LDWEIGHTS")
    if fuse_ldw:
        n = _fuse_ldweights(nc)
        if os.environ.get("ADAAUG_DEBUG"):
            print(f"fused {n} LDWEIGHTS")
    return _legalize_waits(nc) if legalize else nc


def img_like_slice(t, c, k, m):
    """[128,128] lhsT tile: rows h in k-block, cols w/j in m-block of ch c."""
    base = c * 512 + k * 256 + m * 128
    return t[:, base: base + 128]


# --------------------------------------------------------------------------
# Entry point
# --------------------------------------------------------------------------

def _prep_in_maps(images, gates, gauss, unif, noise_img, hz_fbank):
    from concourse import mybir

    prm = derive_params(gates, gauss, unif, hz_fbank)
    fp8_np = mybir.dt.np(mybir.dt.float8e4)
    # color premix folded into the packing/conversion pass:
    # mixed[s,c] = sum_c' M3[s,c,c'] img[s,c'] + b[s,c]
    mixed = np.einsum(
        "scd,sdhw->schw", prm["M3"].astype(np.float32),
        images.astype(np.float32)) + prm["bvec"].astype(np.float32)[:, :, None, None]
    imgs_bf = pack_images(mixed).astype(BF16)
    noiseS = noise_img * prm["sigma"][:, None, None, None].astype(np.float32)
    noise_f8 = pack_images(noiseS).astype(fp8_np)
    w_bf = pack_wmats(prm["Wmats"].astype(np.float32)).astype(BF16)
    mask_f8 = pack_field(prm["mask"]).astype(fp8_np)
    ident_f8 = np.eye(128, dtype=np.float32).astype(fp8_np)
    # one packed uint8 row per (sample, partition):
    #   img bf16 | noise fp8 | W bf16 | mask fp8
    packed = np.concatenate([
        imgs_bf.view(np.uint8),
        noise_f8.view(np.uint8),
        w_bf.view(np.uint8),
        mask_f8.view(np.uint8),
    ], axis=2)
    assert packed.shape == (N, 128, INB), packed.shape
    in_maps = []
    for i in range(NCORES):
        lo, hi = i * NLOC, (i + 1) * NLOC
        in_maps.append({
            "inpack": np.ascontiguousarray(packed[lo:hi]),
            "consts": ident_f8,
        })
    return in_maps, prm


_NC_CACHE = {}


def run_on_hw(images, gates, gauss, unif, noise_img, hz_fbank, trace=False):
    from concourse.bass_utils import run_bass_kernel_spmd

    if "nc" not in _NC_CACHE:
        _NC_CACHE["nc"] = build_bass(
            dedupe_ldw=os.environ.get("ADAAUG_DEDUPE_LDW", "0") == "1")
    nc = _NC_CACHE["nc"]
    in_maps, _ = _prep_in_maps(images, gates, gauss, unif, noise_img, hz_fbank)
    res = run_bass_kernel_spmd(
        nc, in_maps, core_ids=list(range(NCORES)), trace=trace
    )
    out = np.concatenate(
        [unpack_images(np.asarray(r["out"]).astype(np.float32))
         for r in res.results], axis=0
    )
    return out.astype(np.float32), res


def kernel(images, gates, gauss, unif, noise_img, hz_fbank):
    images = np.asarray(images, dtype=np.float32)
    gates = np.asarray(gates, dtype=np.float32)
    gauss = np.asarray(gauss, dtype=np.float32)
    unif = np.asarray(unif, dtype=np.float32)
    noise_img = np.asarray(noise_img, dtype=np.float32)
    hz_fbank = np.asarray(hz_fbank, dtype=np.float32)
    out, _ = run_on_hw(images, gates, gauss, unif, noise_img, hz_fbank,
                       trace=os.environ.get("ADAAUG_TRACE", "0") == "1")
    return out
